# revision 16
# baseline (speedup 1.0000x reference)
"""Multi-head attention (B=4, T=2048, D=1024, H=16) on 8 TRN2 NeuronCores.

Sharding: core c handles batch b = c//2 and head-half hh = c%2 (8 heads,
512 of the 1024 channel dims). Each core computes its half of the head
outputs and a row-sharded output projection, producing a partial
[T, D] output. Host unshard: out[b] = partial[2b] + partial[2b+1]
+ b_o + b_v @ w_o.T (the value-bias contribution commutes through
attention because softmax rows sum to 1).

v7: all-bf16 matmuls, ACT-paced softmax pipeline.
  PSUM: scores 2x[128,1024] double-buffered, av 2x[65,512] (also reused
  for the K=1 denominator-broadcast matmuls), proj 2x[128,512] shared by
  QKV projections and the output projection.
  - Softmax denominator rides row 64 of the av accumulators (ones column
    in V); a K=1 PE matmul broadcasts it across partitions, so the
    normalize chain (reciprocal, scale) never waits on a DMA.
  - t-block 0's attention is chunked by tk-quarters with SBUF
    accumulation, so it streams as each K/V projection t-block lands
    instead of stalling on the full K/V sweep.
  - Out-projection chains of the previous t-block and the next Q
    projection are woven into the attention emission as PE filler.
"""

from contextlib import ExitStack

import numpy as np
import ml_dtypes

import concourse.bass as bass
import concourse.mybir as mybir
import concourse.tile as tile
from concourse import bacc
from concourse.bass_utils import run_bass_kernel_spmd

B, T, D = 4, 2048, 1024
H = 16
DH = 64  # head dim
HALF = 512  # channels per core (8 heads)
N_CORES = 8

F32 = mybir.dt.float32
BF16 = mybir.dt.bfloat16

TB = 512  # t-block for moving operands
NTB = T // TB  # 4
KB = 128  # contraction block
NKB = D // KB  # 8
NJB = HALF // KB  # 4 j-blocks of the half
NTK = T // KB  # 16 tk blocks


def build_kernel():
    nc = bacc.Bacc(
        "TRN2", target_bir_lowering=False, debug=False, num_devices=N_CORES
    )
    xqT = nc.dram_tensor("xqT", [D, T], BF16, kind="ExternalInput").ap()
    xkT = nc.dram_tensor("xkT", [D, T], BF16, kind="ExternalInput").ap()
    xvT = nc.dram_tensor("xvT", [D, T], BF16, kind="ExternalInput").ap()
    wqT = nc.dram_tensor("wqT", [D, HALF], BF16, kind="ExternalInput").ap()
    wkT = nc.dram_tensor("wkT", [D, HALF], BF16, kind="ExternalInput").ap()
    wvT = nc.dram_tensor("wvT", [D, HALF], BF16, kind="ExternalInput").ap()
    woT = nc.dram_tensor("woT", [HALF, D], BF16, kind="ExternalInput").ap()
    bq = nc.dram_tensor("bq", [HALF, 1], F32, kind="ExternalInput").ap()
    bk = nc.dram_tensor("bk", [HALF, 1], F32, kind="ExternalInput").ap()
    ones_in = nc.dram_tensor("ones_in", [KB, H // 2], BF16, kind="ExternalInput").ap()
    ones_bc_in = nc.dram_tensor(
        "ones_bc_in", [DH + 1, DH], BF16, kind="ExternalInput"
    ).ap()
    partial = nc.dram_tensor("partial", [T, D], F32, kind="ExternalOutput").ap()

    with tile.TileContext(nc) as tc, ExitStack() as ctx:
        p_const = ctx.enter_context(tc.tile_pool(name="const", bufs=1))
        p_kt = ctx.enter_context(tc.tile_pool(name="kt", bufs=NJB * NTB))
        p_v = ctx.enter_context(tc.tile_pool(name="v", bufs=NTK))
        p_qt = ctx.enter_context(tc.tile_pool(name="qt", bufs=2 * NJB))
        p_xs = ctx.enter_context(tc.tile_pool(name="xs", bufs=18))
        p_ex = ctx.enter_context(tc.tile_pool(name="ex", bufs=4))
        p_ot = ctx.enter_context(tc.tile_pool(name="ot", bufs=2 * NJB))
        p_as = ctx.enter_context(tc.tile_pool(name="as", bufs=6))
        p_ac = ctx.enter_context(tc.tile_pool(name="ac", bufs=8))
        p_rc = ctx.enter_context(tc.tile_pool(name="rc", bufs=3))
        p_st = ctx.enter_context(tc.tile_pool(name="st", bufs=2))
        # PSUM: scores 2x2 banks + av 2x1 + proj/outproj 2x1 = 8 banks
        p_sc = ctx.enter_context(tc.tile_pool(name="sc", bufs=2, space="PSUM"))
        p_av = ctx.enter_context(tc.tile_pool(name="av", bufs=2, space="PSUM"))
        p_pj = ctx.enter_context(tc.tile_pool(name="pj", bufs=2, space="PSUM"))

        # ---- constants (w_o is emitted last: it is only needed once the
        # first out-projection runs, well after the first K/V chains) ----
        w_k = p_const.tile([KB, NKB, HALF], BF16, tag="wk")
        nc.sync.dma_start(w_k[:], wkT.rearrange("(kb p) j -> p kb j", p=KB))
        b_k = p_const.tile([KB, NJB], F32, tag="bk")
        nc.sync.dma_start(b_k[:], bk.rearrange("(jb p) one -> p (jb one)", p=KB))
        ones8 = p_const.tile([KB, H // 2], BF16, tag="ones8")
        nc.sync.dma_start(ones8[:], ones_in[:])
        # [1, 64] of ones at partition 64: lhsT of the K=1 broadcast matmul
        ones_bc = p_const.tile([DH + 1, DH], BF16, tag="onesbc")
        nc.sync.dma_start(ones_bc[:], ones_bc_in[:])
        w_v = p_const.tile([KB, NKB, HALF], BF16, tag="wv")
        nc.sync.dma_start(w_v[:], wvT.rearrange("(kb p) j -> p kb j", p=KB))
        w_q = p_const.tile([KB, NKB, HALF], BF16, tag="wq")
        nc.sync.dma_start(w_q[:], wqT.rearrange("(kb p) j -> p kb j", p=KB))
        b_q = p_const.tile([KB, NJB], F32, tag="bq")
        nc.sync.dma_start(b_q[:], bq.rearrange("(jb p) one -> p (jb one)", p=KB))
        w_o = p_const.tile([KB, NJB, D], BF16, tag="wo")

        def load_x_tiles(src, tb):
            """DMA one t-block of an input into 8 resident [128, 512] tiles."""
            xts = []
            for kb in range(NKB):
                xt = p_xs.tile([KB, TB], BF16, tag="xs")
                nc.sync.dma_start(
                    xt[:], src[kb * KB : (kb + 1) * KB, tb * TB : (tb + 1) * TB]
                )
                xts.append(xt)
            return xts

        # kt[jb][tb]: [128 (j), TB] tiles (separate tiles per t-block so
        # attention groups depend only on the t-blocks they read)
        kt_tiles = [
            [p_kt.tile([KB, TB], BF16, tag="kt", name=f"kt{j}_{tb}") for tb in range(NTB)]
            for j in range(NJB)
        ]
        v_tiles = [
            p_v.tile([KB, H // 2, DH + 1], BF16, tag="v", name=f"v{j}")
            for j in range(NTK)
        ]

        def kv_proj_chains(tb):
            """K^T + V projection chains for one t-block, as thunks."""
            xk_tiles, xv_tiles = [], []

            def k_chain(jb):
                def emit():
                    if not xk_tiles:
                        xk_tiles.extend(load_x_tiles(xkT, tb))
                    ps = p_pj.tile([KB, TB], F32, tag="pj")
                    for kb in range(NKB):
                        nc.tensor.matmul(
                            ps[:],
                            w_k[:, kb, jb * KB : (jb + 1) * KB],
                            xk_tiles[kb][:],
                            start=(kb == 0),
                            stop=(kb == NKB - 1),
                        )
                    nc.vector.tensor_scalar_add(
                        kt_tiles[jb][tb][:], ps[:], b_k[:, jb : jb + 1]
                    )

                return emit

            def v_chain(ts):
                def emit():
                    if not xv_tiles:
                        for u in range(4):
                            nc.sync.dma_start(
                                v_tiles[tb * 4 + u][:, :, DH : DH + 1],
                                ones8[:, :, None],
                            )
                        xv_tiles.extend(load_x_tiles(xvT, tb))
                    ps = p_pj.tile([KB, TB], F32, tag="pj")
                    for kb in range(NKB):
                        nc.tensor.matmul(
                            ps[:],
                            xv_tiles[kb][:, ts * KB : (ts + 1) * KB],
                            w_v[:, kb, :],
                            start=(kb == 0),
                            stop=(kb == NKB - 1),
                        )
                    nc.vector.tensor_copy(
                        v_tiles[tb * 4 + ts][:, :, 0:DH],
                        ps[:].rearrange("p (h d) -> p h d", d=DH),
                    )

                return emit

            # K chains before V chains: the shared x-tile pool releases the
            # xk residents before the xv loads need slots
            return [k_chain(u) for u in range(4)] + [v_chain(u) for u in range(4)]

        def q_proj(tq):
            qt_tiles = [
                p_qt.tile([KB, TB], BF16, tag="qt", name=f"qt{j}") for j in range(NJB)
            ]
            xts = load_x_tiles(xqT, tq)
            for jb in range(NJB):
                ps = p_pj.tile([KB, TB], F32, tag="pj")
                for kb in range(NKB):
                    nc.tensor.matmul(
                        ps[:],
                        w_q[:, kb, jb * KB : (jb + 1) * KB],
                        xts[kb][:],
                        start=(kb == 0),
                        stop=(kb == NKB - 1),
                    )
                nc.vector.tensor_scalar_add(
                    qt_tiles[jb][:], ps[:], b_q[:, jb : jb + 1]
                )
            return qt_tiles

        def normalize(jp, i, src, ot_tiles):
            """src: [DH+1, TB] f32 (SBUF) accumulated head output; row DH is
            the softmax denominator. PE broadcasts it across partitions via a
            K=1 matmul; reciprocal+scale on DVE; result -> ot pair tile."""
            db = p_as.tile([DH + 1, TB], BF16, tag="db")
            nc.vector.tensor_copy(db[DH : DH + 1, :], src[DH : DH + 1, :])
            dbc = p_av.tile([DH, TB], F32, tag="av", name="dbc")
            nc.tensor.matmul(
                dbc[:],
                ones_bc[DH : DH + 1, :],
                db[DH : DH + 1, :],
                start=True,
                stop=True,
            )
            rc2 = p_rc.tile([DH, TB], F32, tag="rc2")
            nc.vector.reciprocal_approx_fast(rc2[:], dbc[:])
            if i == 0:
                nc.vector.tensor_mul(ot_tiles[jp][0:DH, :], src[0:DH, :], rc2[:])
            else:
                # DVE can't shift partitions; stage then DMA into rows 64:128
                stg = p_rc.tile([DH, TB], BF16, tag="stg")
                nc.vector.tensor_mul(stg[:], src[0:DH, :], rc2[:])
                nc.sync.dma_start(ot_tiles[jp][DH : 2 * DH, :], stg[:])

        def attention(qt_tiles, chunks, chunk_pre=None, filler=(), jp_post=None):
            """One t-block of attention over tk chunks; returns ot pair-tiles.

            chunks: list of tk-index lists. Single chunk: accumulate in psum
            and normalize straight from it. Multiple chunks: spill/add each
            chunk into an SBUF accumulator (so attention streams while later
            K/V t-blocks are still being projected).
            chunk_pre: {chunk_idx: [thunks]} emitted before that chunk.
            filler: thunks woven in after each head pair's normalize.
            jp_post: {jp: [thunks]} emitted after that pair's normalize.
            """
            chunk_pre = chunk_pre or {}
            jp_post = jp_post or {}
            filler = list(filler)
            chunked = len(chunks) > 1
            ot_tiles = [
                p_ot.tile([KB, TB], BF16, tag="ot", name=f"ot{j}") for j in range(NJB)
            ]
            acc = {}
            if chunked:
                for jp in range(NJB):
                    for i in range(2):
                        acc[jp, i] = p_ac.tile(
                            [DH + 1, TB], F32, tag="ac", name=f"ac{jp}_{i}"
                        )

            for ci, chunk in enumerate(chunks):
                for thunk in chunk_pre.get(ci, []):
                    thunk()
                last_chunk = ci == len(chunks) - 1
                for jp in range(NJB):  # head pair (2*jp, 2*jp+1)
                    avs = [
                        p_av.tile([DH + 1, TB], F32, tag="av", name=f"av{i}")
                        for i in range(2)
                    ]
                    for tk in chunk:
                        sc = p_sc.tile([KB, 2 * TB], F32, tag="sc")
                        # scores: the two matmuls hit row groups 0/64 and run
                        # concurrently in the PE array
                        for i in range(2):
                            nc.tensor.matmul(
                                sc[:, i * TB : (i + 1) * TB],
                                kt_tiles[jp][tk // 4][
                                    i * DH : (i + 1) * DH,
                                    (tk % 4) * KB : (tk % 4 + 1) * KB,
                                ],
                                qt_tiles[jp][i * DH : (i + 1) * DH, :],
                                start=True,
                                stop=True,
                            )
                        ex = p_ex.tile([KB, 2 * TB], BF16, tag="ex")
                        nc.scalar.activation(
                            ex[:], sc[:], mybir.ActivationFunctionType.Exp, scale=0.125
                        )
                        for i in range(2):
                            nc.tensor.matmul(
                                avs[i][:],
                                v_tiles[tk][:, 2 * jp + i, :],
                                ex[:, i * TB : (i + 1) * TB],
                                start=(tk == chunk[0]),
                                stop=(tk == chunk[-1]),
                            )
                    if chunked:
                        for i in range(2):
                            if ci == 0:
                                nc.vector.tensor_copy(acc[jp, i][:], avs[i][:])
                            else:
                                nc.vector.tensor_add(
                                    acc[jp, i][:], acc[jp, i][:], avs[i][:]
                                )
                        if last_chunk:
                            for i in (1, 0):
                                normalize(jp, i, acc[jp, i], ot_tiles)
                    else:
                        av_s = []
                        for i in range(2):
                            a = p_as.tile([DH + 1, TB], F32, tag="as")
                            nc.vector.tensor_copy(a[:], avs[i][:])
                            av_s.append(a)
                        for i in (1, 0):
                            normalize(jp, i, av_s[i], ot_tiles)
                    if last_chunk:
                        for thunk in jp_post.get(jp, []):
                            thunk()
                        take = (
                            len(filler) // (NJB - jp)
                            if jp < NJB - 1
                            else len(filler)
                        )
                        for _ in range(take):
                            filler.pop(0)()
            return ot_tiles

        def out_proj_chains(tq, ot_tiles):
            def chain(nb, ts):
                def emit():
                    po = p_pj.tile([KB, TB], F32, tag="pj")
                    for jp in range(NJB):
                        nc.tensor.matmul(
                            po[:],
                            ot_tiles[jp][:, ts * KB : (ts + 1) * KB],
                            w_o[:, jp, nb * TB : (nb + 1) * TB],
                            start=(jp == 0),
                            stop=(jp == NJB - 1),
                        )
                    st = p_st.tile([KB, TB], F32, tag="st")
                    nc.vector.tensor_copy(st[:], po[:])
                    nc.sync.dma_start(
                        partial[
                            tq * TB + ts * KB : tq * TB + (ts + 1) * KB,
                            nb * TB : (nb + 1) * TB,
                        ],
                        st[:],
                    )

                return emit

            return [chain(nb, ts) for nb in range(2) for ts in range(4)]

        # ---- emission ----
        for thunk in kv_proj_chains(0):
            thunk()
        qt = q_proj(0)

        qt_next = []

        def q_thunk(tq):
            def emit():
                qt_next.append(q_proj(tq))

            return emit

        pending = []  # out-projection chains of the previous t-block
        for tq in range(NTB):
            if tq == 0:
                chunks = [list(range(c * 4, c * 4 + 4)) for c in range(4)]
                pre = {c: kv_proj_chains(c) for c in range(1, 4)}
                pre[3] = pre[3] + [
                    lambda: nc.sync.dma_start(
                        w_o[:], woT.rearrange("(jb p) n -> p jb n", p=KB)
                    )
                ]
            else:
                chunks = [list(range(NTK))]
                pre = {}
            qt_next.clear()
            ot = attention(
                qt,
                chunks,
                chunk_pre=pre,
                filler=pending,
                jp_post={2: [q_thunk(tq + 1)]} if tq + 1 < NTB else {},
            )
            if tq + 1 < NTB:
                qt = qt_next[0]
            pending = out_proj_chains(tq, ot)
        for c in pending:
            c()

    nc.compile()
    return nc


def kernel(**inputs: np.ndarray) -> np.ndarray:
    query = np.asarray(inputs["query"], dtype=np.float32)
    key = np.asarray(inputs["key"], dtype=np.float32)
    value = np.asarray(inputs["value"], dtype=np.float32)
    w_q = np.asarray(inputs["w_q"], dtype=np.float32)
    b_q = np.asarray(inputs["b_q"], dtype=np.float32)
    w_k = np.asarray(inputs["w_k"], dtype=np.float32)
    b_k = np.asarray(inputs["b_k"], dtype=np.float32)
    w_v = np.asarray(inputs["w_v"], dtype=np.float32)
    b_v = np.asarray(inputs["b_v"], dtype=np.float32)
    w_o = np.asarray(inputs["w_o"], dtype=np.float32)
    b_o = np.asarray(inputs["b_o"], dtype=np.float32)

    nc = build_kernel()

    bf = ml_dtypes.bfloat16
    in_maps = []
    for c in range(N_CORES):
        b = c // 2
        hh = c % 2
        sl = slice(hh * HALF, (hh + 1) * HALF)
        in_maps.append(
            {
                "xqT": np.ascontiguousarray(query[b].T.astype(bf)),
                "xkT": np.ascontiguousarray(key[b].T.astype(bf)),
                "xvT": np.ascontiguousarray(value[b].T.astype(bf)),
                "wqT": np.ascontiguousarray(w_q[sl, :].T.astype(bf)),
                "wkT": np.ascontiguousarray(w_k[sl, :].T.astype(bf)),
                "wvT": np.ascontiguousarray(w_v[sl, :].T.astype(bf)),
                "woT": np.ascontiguousarray(w_o[:, sl].T.astype(bf)),
                "bq": np.ascontiguousarray(b_q[sl].reshape(HALF, 1)),
                "bk": np.ascontiguousarray(b_k[sl].reshape(HALF, 1)),
                "ones_in": np.ones((KB, H // 2), dtype=bf),
                "ones_bc_in": np.ones((DH + 1, DH), dtype=bf),
            }
        )

    res = run_bass_kernel_spmd(nc, in_maps, core_ids=list(range(N_CORES)))

    const_row = (b_v[None, :] @ w_o.T + b_o[None, :]).astype(np.float32)
    out = np.empty((B, T, D), dtype=np.float32)
    for b in range(B):
        out[b] = res.results[2 * b]["partial"] + res.results[2 * b + 1]["partial"]
        out[b] += const_row
    return out


# revision 17
# speedup vs baseline: 1.1345x; 1.1345x over previous
"""Multi-head attention (B=4, T=2048, D=1024, H=16) on 8 TRN2 NeuronCores.

Sharding: core c handles batch b = c//2 and head-half hh = c%2 (8 heads,
512 of the 1024 channel dims). Each core computes its half of the head
outputs and a row-sharded output projection, producing a partial
[T, D] output. Host unshard: out[b] = partial[2b] + partial[2b+1]
+ b_o + b_v @ w_o.T (the value-bias contribution commutes through
attention because softmax rows sum to 1).

v7: all-bf16 matmuls, ACT-paced softmax pipeline.
  PSUM: scores 2x[128,1024] double-buffered, av 2x[65,512] (also reused
  for the K=1 denominator-broadcast matmuls), proj 2x[128,512] shared by
  QKV projections and the output projection.
  - Softmax denominator rides row 64 of the av accumulators (ones column
    in V); a K=1 PE matmul broadcasts it across partitions, so the
    normalize chain (reciprocal, scale) never waits on a DMA.
  - t-block 0's attention is chunked by tk-quarters with SBUF
    accumulation, so it streams as each K/V projection t-block lands
    instead of stalling on the full K/V sweep.
  - Out-projection chains of the previous t-block and the next Q
    projection are woven into the attention emission as PE filler.
"""

from contextlib import ExitStack

import numpy as np
import ml_dtypes

import concourse.bass as bass
import concourse.mybir as mybir
import concourse.tile as tile
from concourse import bacc
from concourse.bass_utils import run_bass_kernel_spmd

B, T, D = 4, 2048, 1024
H = 16
DH = 64  # head dim
HALF = 512  # channels per core (8 heads)
N_CORES = 8

F32 = mybir.dt.float32
BF16 = mybir.dt.bfloat16

TB = 512  # t-block for moving operands
NTB = T // TB  # 4
KB = 128  # contraction block
NKB = D // KB  # 8
NJB = HALF // KB  # 4 j-blocks of the half
NTK = T // KB  # 16 tk blocks


def build_kernel():
    nc = bacc.Bacc(
        "TRN2", target_bir_lowering=False, debug=False, num_devices=N_CORES
    )
    xqT = nc.dram_tensor("xqT", [D, T], BF16, kind="ExternalInput").ap()
    xkT = nc.dram_tensor("xkT", [D, T], BF16, kind="ExternalInput").ap()
    xvT = nc.dram_tensor("xvT", [D, T], BF16, kind="ExternalInput").ap()
    wqT = nc.dram_tensor("wqT", [D, HALF], BF16, kind="ExternalInput").ap()
    wkT = nc.dram_tensor("wkT", [D, HALF], BF16, kind="ExternalInput").ap()
    wvT = nc.dram_tensor("wvT", [D, HALF], BF16, kind="ExternalInput").ap()
    woT = nc.dram_tensor("woT", [HALF, D], BF16, kind="ExternalInput").ap()
    bq = nc.dram_tensor("bq", [HALF, 1], F32, kind="ExternalInput").ap()
    bk = nc.dram_tensor("bk", [HALF, 1], F32, kind="ExternalInput").ap()
    ones_in = nc.dram_tensor("ones_in", [KB, H // 2], BF16, kind="ExternalInput").ap()
    ones_bc_in = nc.dram_tensor(
        "ones_bc_in", [DH + 1, DH], BF16, kind="ExternalInput"
    ).ap()
    partial = nc.dram_tensor("partial", [T, D], F32, kind="ExternalOutput").ap()

    with tile.TileContext(nc) as tc, ExitStack() as ctx:
        p_const = ctx.enter_context(tc.tile_pool(name="const", bufs=1))
        p_kt = ctx.enter_context(tc.tile_pool(name="kt", bufs=NJB * NTB))
        p_v = ctx.enter_context(tc.tile_pool(name="v", bufs=NTK))
        p_qt = ctx.enter_context(tc.tile_pool(name="qt", bufs=2 * NJB))
        p_xs = ctx.enter_context(tc.tile_pool(name="xs", bufs=18))
        p_ex = ctx.enter_context(tc.tile_pool(name="ex", bufs=4))
        p_ot = ctx.enter_context(tc.tile_pool(name="ot", bufs=2 * NJB))
        p_as = ctx.enter_context(tc.tile_pool(name="as", bufs=6))
        p_ac = ctx.enter_context(tc.tile_pool(name="ac", bufs=8))
        p_rc = ctx.enter_context(tc.tile_pool(name="rc", bufs=3))
        p_st = ctx.enter_context(tc.tile_pool(name="st", bufs=2))
        # PSUM: scores 2x2 banks + av 2x1 + proj/outproj 2x1 = 8 banks
        p_sc = ctx.enter_context(tc.tile_pool(name="sc", bufs=2, space="PSUM"))
        p_av = ctx.enter_context(tc.tile_pool(name="av", bufs=2, space="PSUM"))
        p_pj = ctx.enter_context(tc.tile_pool(name="pj", bufs=2, space="PSUM"))

        # ---- constants (w_o is emitted last: it is only needed once the
        # first out-projection runs, well after the first K/V chains) ----
        w_k = p_const.tile([KB, NKB, HALF], BF16, tag="wk")
        nc.sync.dma_start(w_k[:], wkT.rearrange("(kb p) j -> p kb j", p=KB))
        b_k = p_const.tile([KB, NJB], F32, tag="bk")
        nc.sync.dma_start(b_k[:], bk.rearrange("(jb p) one -> p (jb one)", p=KB))
        ones8 = p_const.tile([KB, H // 2], BF16, tag="ones8")
        nc.sync.dma_start(ones8[:], ones_in[:])
        # [1, 64] of ones at partition 64: lhsT of the K=1 broadcast matmul
        ones_bc = p_const.tile([DH + 1, DH], BF16, tag="onesbc")
        nc.sync.dma_start(ones_bc[:], ones_bc_in[:])
        w_v = p_const.tile([KB, NKB, HALF], BF16, tag="wv")
        nc.sync.dma_start(w_v[:], wvT.rearrange("(kb p) j -> p kb j", p=KB))
        w_q = p_const.tile([KB, NKB, HALF], BF16, tag="wq")
        nc.sync.dma_start(w_q[:], wqT.rearrange("(kb p) j -> p kb j", p=KB))
        b_q = p_const.tile([KB, NJB], F32, tag="bq")
        nc.sync.dma_start(b_q[:], bq.rearrange("(jb p) one -> p (jb one)", p=KB))
        w_o = p_const.tile([KB, NJB, D], BF16, tag="wo")

        def load_x_tiles(src, tb):
            """DMA one t-block of an input into 8 resident [128, 512] tiles."""
            xts = []
            for kb in range(NKB):
                xt = p_xs.tile([KB, TB], BF16, tag="xs")
                nc.sync.dma_start(
                    xt[:], src[kb * KB : (kb + 1) * KB, tb * TB : (tb + 1) * TB]
                )
                xts.append(xt)
            return xts

        # kt[jb][tb]: [128 (j), TB] tiles (separate tiles per t-block so
        # attention groups depend only on the t-blocks they read)
        kt_tiles = [
            [p_kt.tile([KB, TB], BF16, tag="kt", name=f"kt{j}_{tb}") for tb in range(NTB)]
            for j in range(NJB)
        ]
        v_tiles = [
            p_v.tile([KB, H // 2, DH + 1], BF16, tag="v", name=f"v{j}")
            for j in range(NTK)
        ]

        def kv_proj_chains(tb):
            """K^T + V projection chains for one t-block, as thunks."""
            xk_tiles, xv_tiles = [], []

            def k_chain(jb):
                def emit():
                    if not xk_tiles:
                        xk_tiles.extend(load_x_tiles(xkT, tb))
                    ps = p_pj.tile([KB, TB], F32, tag="pj")
                    for kb in range(NKB):
                        nc.tensor.matmul(
                            ps[:],
                            w_k[:, kb, jb * KB : (jb + 1) * KB],
                            xk_tiles[kb][:],
                            start=(kb == 0),
                            stop=(kb == NKB - 1),
                        )
                    nc.vector.tensor_scalar_add(
                        kt_tiles[jb][tb][:], ps[:], b_k[:, jb : jb + 1]
                    )

                return emit

            def v_chain(ts):
                def emit():
                    if not xv_tiles:
                        for u in range(4):
                            nc.sync.dma_start(
                                v_tiles[tb * 4 + u][:, :, DH : DH + 1],
                                ones8[:, :, None],
                            )
                        xv_tiles.extend(load_x_tiles(xvT, tb))
                    ps = p_pj.tile([KB, TB], F32, tag="pj")
                    for kb in range(NKB):
                        nc.tensor.matmul(
                            ps[:],
                            xv_tiles[kb][:, ts * KB : (ts + 1) * KB],
                            w_v[:, kb, :],
                            start=(kb == 0),
                            stop=(kb == NKB - 1),
                        )
                    nc.vector.tensor_copy(
                        v_tiles[tb * 4 + ts][:, :, 0:DH],
                        ps[:].rearrange("p (h d) -> p h d", d=DH),
                    )

                return emit

            # K chains before V chains: the shared x-tile pool releases the
            # xk residents before the xv loads need slots
            return [k_chain(u) for u in range(4)] + [v_chain(u) for u in range(4)]

        def q_proj(tq):
            qt_tiles = [
                p_qt.tile([KB, TB], BF16, tag="qt", name=f"qt{j}") for j in range(NJB)
            ]
            xts = load_x_tiles(xqT, tq)
            for jb in range(NJB):
                ps = p_pj.tile([KB, TB], F32, tag="pj")
                for kb in range(NKB):
                    nc.tensor.matmul(
                        ps[:],
                        w_q[:, kb, jb * KB : (jb + 1) * KB],
                        xts[kb][:],
                        start=(kb == 0),
                        stop=(kb == NKB - 1),
                    )
                nc.vector.tensor_scalar_add(
                    qt_tiles[jb][:], ps[:], b_q[:, jb : jb + 1]
                )
            return qt_tiles

        def normalize(jp, i, src, ot_tiles):
            """src: [DH+1, TB] f32 (SBUF) accumulated head output; row DH is
            the softmax denominator. PE broadcasts it across partitions via a
            K=1 matmul; reciprocal+scale on DVE; result -> ot pair tile."""
            db = p_as.tile([DH + 1, TB], BF16, tag="db")
            nc.vector.tensor_copy(db[DH : DH + 1, :], src[DH : DH + 1, :])
            dbc = p_av.tile([DH, TB], F32, tag="av", name="dbc")
            nc.tensor.matmul(
                dbc[:],
                ones_bc[DH : DH + 1, :],
                db[DH : DH + 1, :],
                start=True,
                stop=True,
            )
            rc2 = p_rc.tile([DH, TB], F32, tag="rc2")
            nc.vector.reciprocal_approx_fast(rc2[:], dbc[:])
            if i == 0:
                nc.vector.tensor_mul(ot_tiles[jp][0:DH, :], src[0:DH, :], rc2[:])
            else:
                # DVE can't shift partitions; stage then DMA into rows 64:128
                stg = p_rc.tile([DH, TB], BF16, tag="stg")
                nc.vector.tensor_mul(stg[:], src[0:DH, :], rc2[:])
                nc.sync.dma_start(ot_tiles[jp][DH : 2 * DH, :], stg[:])

        def attention(qt_tiles, chunks, chunk_pre=None, filler=(), jp_post=None):
            """One t-block of attention over tk chunks; returns ot pair-tiles.

            chunks: list of tk-index lists. Single chunk: accumulate in psum
            and normalize straight from it. Multiple chunks: spill/add each
            chunk into an SBUF accumulator (so attention streams while later
            K/V t-blocks are still being projected).
            chunk_pre: {chunk_idx: [thunks]} emitted before that chunk.
            filler: thunks woven in after each head pair's normalize.
            jp_post: {jp: [thunks]} emitted after that pair's normalize.
            """
            chunk_pre = chunk_pre or {}
            jp_post = jp_post or {}
            filler = list(filler)
            chunked = len(chunks) > 1
            ot_tiles = [
                p_ot.tile([KB, TB], BF16, tag="ot", name=f"ot{j}") for j in range(NJB)
            ]
            acc = {}
            if chunked:
                for jp in range(NJB):
                    for i in range(2):
                        acc[jp, i] = p_ac.tile(
                            [DH + 1, TB], F32, tag="ac", name=f"ac{jp}_{i}"
                        )

            for ci, chunk in enumerate(chunks):
                for thunk in chunk_pre.get(ci, []):
                    thunk()
                last_chunk = ci == len(chunks) - 1
                for jp in range(NJB):  # head pair (2*jp, 2*jp+1)
                    avs = [
                        p_av.tile([DH + 1, TB], F32, tag="av", name=f"av{i}")
                        for i in range(2)
                    ]
                    for tk in chunk:
                        sc = p_sc.tile([KB, 2 * TB], F32, tag="sc")
                        # scores: the two matmuls hit row groups 0/64 and run
                        # concurrently in the PE array
                        for i in range(2):
                            nc.tensor.matmul(
                                sc[:, i * TB : (i + 1) * TB],
                                kt_tiles[jp][tk // 4][
                                    i * DH : (i + 1) * DH,
                                    (tk % 4) * KB : (tk % 4 + 1) * KB,
                                ],
                                qt_tiles[jp][i * DH : (i + 1) * DH, :],
                                start=True,
                                stop=True,
                            )
                        ex = p_ex.tile([KB, 2 * TB], BF16, tag="ex")
                        nc.scalar.activation(
                            ex[:], sc[:], mybir.ActivationFunctionType.Exp, scale=0.125
                        )
                        for i in range(2):
                            nc.tensor.matmul(
                                avs[i][:],
                                v_tiles[tk][:, 2 * jp + i, :],
                                ex[:, i * TB : (i + 1) * TB],
                                start=(tk == chunk[0]),
                                stop=(tk == chunk[-1]),
                            )
                    if chunked:
                        for i in range(2):
                            if ci == 0:
                                nc.vector.tensor_copy(acc[jp, i][:], avs[i][:])
                            else:
                                nc.vector.tensor_add(
                                    acc[jp, i][:], acc[jp, i][:], avs[i][:]
                                )
                        if last_chunk:
                            for i in (1, 0):
                                normalize(jp, i, acc[jp, i], ot_tiles)
                    else:
                        av_s = []
                        for i in range(2):
                            a = p_as.tile([DH + 1, TB], F32, tag="as")
                            nc.vector.tensor_copy(a[:], avs[i][:])
                            av_s.append(a)
                        for i in (1, 0):
                            normalize(jp, i, av_s[i], ot_tiles)
                    if last_chunk:
                        for thunk in jp_post.get(jp, []):
                            thunk()
                        take = (
                            len(filler) // (NJB - jp)
                            if jp < NJB - 1
                            else len(filler)
                        )
                        for _ in range(take):
                            filler.pop(0)()
            return ot_tiles

        def out_proj_chains(tq, ot_tiles):
            def chain(nb, ts):
                def emit():
                    po = p_pj.tile([KB, TB], F32, tag="pj")
                    for jp in range(NJB):
                        nc.tensor.matmul(
                            po[:],
                            ot_tiles[jp][:, ts * KB : (ts + 1) * KB],
                            w_o[:, jp, nb * TB : (nb + 1) * TB],
                            start=(jp == 0),
                            stop=(jp == NJB - 1),
                        )
                    st = p_st.tile([KB, TB], F32, tag="st")
                    nc.vector.tensor_copy(st[:], po[:])
                    nc.sync.dma_start(
                        partial[
                            tq * TB + ts * KB : tq * TB + (ts + 1) * KB,
                            nb * TB : (nb + 1) * TB,
                        ],
                        st[:],
                    )

                return emit

            return [chain(nb, ts) for nb in range(2) for ts in range(4)]

        # ---- emission ----
        for thunk in kv_proj_chains(0):
            thunk()
        qt = q_proj(0)

        qt_next = []

        def q_thunk(tq):
            def emit():
                qt_next.append(q_proj(tq))

            return emit

        for tb in range(1, NTB):
            for thunk in kv_proj_chains(tb):
                thunk()
        nc.sync.dma_start(w_o[:], woT.rearrange("(jb p) n -> p jb n", p=KB))

        pending = []  # out-projection chains of the previous t-block
        for tq in range(NTB):
            qt_next.clear()
            ot = attention(
                qt,
                [list(range(NTK))],
                filler=pending,
                jp_post={2: [q_thunk(tq + 1)]} if tq + 1 < NTB else {},
            )
            if tq + 1 < NTB:
                qt = qt_next[0]
            pending = out_proj_chains(tq, ot)
        for c in pending:
            c()

    nc.compile()
    return nc


def kernel(**inputs: np.ndarray) -> np.ndarray:
    query = np.asarray(inputs["query"], dtype=np.float32)
    key = np.asarray(inputs["key"], dtype=np.float32)
    value = np.asarray(inputs["value"], dtype=np.float32)
    w_q = np.asarray(inputs["w_q"], dtype=np.float32)
    b_q = np.asarray(inputs["b_q"], dtype=np.float32)
    w_k = np.asarray(inputs["w_k"], dtype=np.float32)
    b_k = np.asarray(inputs["b_k"], dtype=np.float32)
    w_v = np.asarray(inputs["w_v"], dtype=np.float32)
    b_v = np.asarray(inputs["b_v"], dtype=np.float32)
    w_o = np.asarray(inputs["w_o"], dtype=np.float32)
    b_o = np.asarray(inputs["b_o"], dtype=np.float32)

    nc = build_kernel()

    bf = ml_dtypes.bfloat16
    in_maps = []
    for c in range(N_CORES):
        b = c // 2
        hh = c % 2
        sl = slice(hh * HALF, (hh + 1) * HALF)
        in_maps.append(
            {
                "xqT": np.ascontiguousarray(query[b].T.astype(bf)),
                "xkT": np.ascontiguousarray(key[b].T.astype(bf)),
                "xvT": np.ascontiguousarray(value[b].T.astype(bf)),
                "wqT": np.ascontiguousarray(w_q[sl, :].T.astype(bf)),
                "wkT": np.ascontiguousarray(w_k[sl, :].T.astype(bf)),
                "wvT": np.ascontiguousarray(w_v[sl, :].T.astype(bf)),
                "woT": np.ascontiguousarray(w_o[:, sl].T.astype(bf)),
                "bq": np.ascontiguousarray(b_q[sl].reshape(HALF, 1)),
                "bk": np.ascontiguousarray(b_k[sl].reshape(HALF, 1)),
                "ones_in": np.ones((KB, H // 2), dtype=bf),
                "ones_bc_in": np.ones((DH + 1, DH), dtype=bf),
            }
        )

    res = run_bass_kernel_spmd(nc, in_maps, core_ids=list(range(N_CORES)))

    const_row = (b_v[None, :] @ w_o.T + b_o[None, :]).astype(np.float32)
    out = np.empty((B, T, D), dtype=np.float32)
    for b in range(B):
        out[b] = res.results[2 * b]["partial"] + res.results[2 * b + 1]["partial"]
        out[b] += const_row
    return out
